# revision 8
# baseline (speedup 1.0000x reference)
"""Causal self-attention block (QKV proj + causal MHA + out proj + residual
+ LayerNorm) for B=4, S=2048, HID=1024, 16 heads, on 8 Trainium2 cores.

Sharding: core c handles batch b=c//2 and heads [8h, 8h+8) where h=c%2
(Megatron-style head split within a batch pair). Each core computes its 8
heads' attention and a partial output projection over the full 2048 rows;
the two cores of a batch pair combine partials with pairwise bf16
ReduceScatters (chunked, pipelined with compute; the final tile uses 4
finer chunks to drain the tail), then each core applies residual +
LayerNorm to its quarter-rows and returns [1024, 1024].

All matmuls run in bf16 (fp32 PSUM accumulation). Attention uses the
transposed-score layout (scoresT[sk, sq]): softmax sums fall out of the
PV matmul via an appended ones-row on V, causal structure shrinks
above-diagonal tiles, and each head pair shares fused two-bank PSUM
tiles so one ACT exp covers both heads; the score matmul for tile i+1 is
emitted ahead of PV(i) so the PE never waits on the exp. The Scalar
engine runs only Exp/Identity/Copy (single activation table, no
reloads); the LN rsqrt is computed on the Vector engine via
reciprocal seed + Newton iterations. Projection work for tile t+1 and
the out projection for tile j-1 are interleaved into attention tile j's
emission to keep the PE dense (p-state) and busy during softmax
normalization windows; LayerNorm chunks are deferred until well after
their ReduceScatter fires, use per-chunk scatter tensors (exact deps),
and all LN DMAs ride the sync queue so collective latency never blocks
the gpsimd queue feeding attention.
"""

import numpy as np
import ml_dtypes

import concourse.bacc as bacc
import concourse.mybir as mybir
import concourse.tile as tile
from concourse.bass_utils import run_bass_kernel_spmd

F32 = mybir.dt.float32
BF16 = mybir.dt.bfloat16
AF = mybir.ActivationFunctionType
OP = mybir.AluOpType
BFNP = ml_dtypes.bfloat16

N_CORES = 8
B, S, HID = 4, 2048, 1024
NHC = 8          # heads per core
DH = 64          # head dim
HW = 512         # per-core head width (NHC * DH)
SQT = 512        # sq tile width
NSQT = S // SQT  # 4
NHCH = HID // 128  # 8 hid chunks
SH = S // 2      # rows per core in the epilogue
EPS = 1e-5
GROUPS = [[0, 1], [2, 3], [4, 5], [6, 7]]

_CACHE = {}


def _build(apply_gb):
    nc = bacc.Bacc("TRN2", target_bir_lowering=False, debug=False,
                   num_devices=N_CORES)

    xst_d = [nc.dram_tensor(f"xst{t}", [128, 8 * SQT], BF16,
                            kind="ExternalInput").ap() for t in range(NSQT)]
    xh = nc.dram_tensor("xh", [SH, HID], F32, kind="ExternalInput").ap()
    wqs_d = nc.dram_tensor("wqs", [128, 8 * HW], BF16,
                           kind="ExternalInput").ap()
    wks_d = nc.dram_tensor("wks", [128, 8 * HW], BF16,
                           kind="ExternalInput").ap()
    wvs_d = nc.dram_tensor("wvs", [128, 8 * HW], BF16,
                           kind="ExternalInput").ap()
    wos_d = nc.dram_tensor("wos", [128, 4 * HID], BF16,
                           kind="ExternalInput").ap()
    bq4 = nc.dram_tensor("bq4", [128, 4], F32, kind="ExternalInput").ap()
    bk4 = nc.dram_tensor("bk4", [128, 4], F32, kind="ExternalInput").ap()
    bv1 = nc.dram_tensor("bv1", [1, HW], BF16, kind="ExternalInput").ap()
    one1 = nc.dram_tensor("one1", [1, 128], BF16, kind="ExternalInput").ap()
    vone = nc.dram_tensor("vone", [128, 8], BF16, kind="ExternalInput").ap()
    m128 = nc.dram_tensor("m128", [128, 128], BF16, kind="ExternalInput").ap()
    gmb = nc.dram_tensor("gmb", [128, HID], F32, kind="ExternalInput").ap()
    btb = nc.dram_tensor("btb", [128, HID], F32, kind="ExternalInput").ap()

    out = nc.dram_tensor("out", [SH, HID], F32, kind="ExternalOutput").ap()

    po_d = nc.dram_tensor("po_d", [S, HID], BF16).ap()
    # per-chunk scatter outputs so LayerNorm dma deps are exact
    rsd = [nc.dram_tensor(f"rs{k}", [128, HID], BF16).ap() for k in range(6)]
    rs3 = [nc.dram_tensor(f"rs3_{c}", [64, HID], BF16).ap() for c in range(4)]

    from contextlib import ExitStack
    with tile.TileContext(nc) as tc, ExitStack() as es:
        TP = tc.tile_pool
        cp = es.enter_context(TP(name="consts", bufs=1))
        xsp = es.enter_context(TP(name="xs", bufs=1))
        wp = es.enter_context(TP(name="w", bufs=1))
        ktp = es.enter_context(TP(name="kt", bufs=1))
        vtp = es.enter_context(TP(name="vt", bufs=1))
        qtp = es.enter_context(TP(name="qt", bufs=2))
        ep = es.enter_context(TP(name="exp", bufs=2))
        atp = es.enter_context(TP(name="att", bufs=2))
        pop = es.enter_context(TP(name="po", bufs=2))
        rp = es.enter_context(TP(name="rcp", bufs=2))
        rbp = es.enter_context(TP(name="rb", bufs=2))
        lp = es.enter_context(TP(name="ln", bufs=2))
        lsp = es.enter_context(TP(name="lns", bufs=2))
        pp = es.enter_context(TP(name="pp", bufs=2, space="PSUM"))
        sp = es.enter_context(TP(name="sp", bufs=2, space="PSUM"))
        app = es.enter_context(TP(name="ap", bufs=1, space="PSUM"))

        # ---- staged preload: one DMA per weight group / x tile, spread
        # across queues so issue cost doesn't serialize ----
        wqs = wp.tile([128, 8 * HW], BF16, name="wqs")
        nc.sync.dma_start(wqs[:], wqs_d[:])
        xst = [xsp.tile([128, 8 * SQT], BF16, name=f"xst{t}")
               for t in range(NSQT)]
        nc.gpsimd.dma_start(xst[0][:], xst_d[0][:])
        wks = wp.tile([128, 8 * HW], BF16, name="wks")
        nc.scalar.dma_start(wks[:], wks_d[:])
        nc.gpsimd.dma_start(xst[1][:], xst_d[1][:])
        wvs = wp.tile([128, 8 * HW], BF16, name="wvs")
        nc.sync.dma_start(wvs[:], wvs_d[:])
        nc.sync.dma_start(xst[2][:], xst_d[2][:])
        wos = wp.tile([128, 4 * HID], BF16, name="wos")
        nc.scalar.dma_start(wos[:], wos_d[:])
        nc.gpsimd.dma_start(xst[3][:], xst_d[3][:])

        # ---- constants ----
        mask = cp.tile([128, 128], BF16)
        nc.sync.dma_start(mask[:], m128[:])
        bqs = cp.tile([128, 4], F32)
        nc.sync.dma_start(bqs[:], bq4[:])
        bks = cp.tile([128, 4], F32)
        nc.sync.dma_start(bks[:], bk4[:])
        bvs = cp.tile([1, HW], BF16)
        nc.sync.dma_start(bvs[:], bv1[:])
        o1s = cp.tile([1, 128], BF16)
        nc.sync.dma_start(o1s[:], one1[:])
        vos = cp.tile([128, 8], BF16)
        nc.sync.dma_start(vos[:], vone[:])
        epsc = cp.tile([128, 1], F32)
        nc.vector.memset(epsc[:], EPS)
        if apply_gb:
            gms = cp.tile([128, HID], F32)
            nc.sync.dma_start(gms[:], gmb[:])
            bts = cp.tile([128, HID], F32)
            nc.sync.dma_start(bts[:], btb[:])

        kt = [ktp.tile([128, S], BF16, name=f"kt{p}") for p in range(4)]
        vt = [vtp.tile([128, 8, 65], BF16, name=f"vt{i}") for i in range(16)]
        for i in range(16):
            nc.vector.tensor_copy(
                vt[i][:, :, 64:65],
                vos[:].rearrange("p (a b) -> p a b", a=8))

        qts_map = {}
        at_map = {}

        def wsl(ws, hh):
            return ws[:, HW * hh:HW * (hh + 1)]

        def xsl(t, hh, c0, w):
            return xst[t][:, SQT * hh + c0:SQT * hh + c0 + w]

        # ---- phase-A units: projections for sq tile t ----
        def unit_q(t, m):
            ps = pp.tile([128, SQT], F32, tag="pq")
            for hh in range(NHCH):
                nc.tensor.matmul(
                    ps[:], wsl(wqs, hh)[:, 128 * m:128 * (m + 1)],
                    xsl(t, hh, 0, SQT),
                    start=(hh == 0), stop=(hh == NHCH - 1))
            qt_ = qtp.tile([128, SQT], BF16, tag=f"q{m}")
            nc.scalar.activation(qt_[:], ps[:], AF.Identity,
                                 bias=bqs[:, m:m + 1])
            qts_map[(t, m)] = qt_

        def unit_k(t, m):
            ps = pp.tile([128, SQT], F32, tag="pq")
            for hh in range(NHCH):
                nc.tensor.matmul(
                    ps[:], wsl(wks, hh)[:, 128 * m:128 * (m + 1)],
                    xsl(t, hh, 0, SQT),
                    start=(hh == 0), stop=(hh == NHCH - 1))
            nc.scalar.activation(kt[m][:, SQT * t:SQT * (t + 1)], ps[:],
                                 AF.Identity, bias=bks[:, m:m + 1])

        def unit_v(t, s_):
            i = 4 * t + s_
            ps = pp.tile([128, HW], F32, tag="pq")
            for hh in range(NHCH):
                nc.tensor.matmul(
                    ps[:], xsl(t, hh, 128 * s_, 128), wsl(wvs, hh),
                    start=(hh == 0), stop=False)
            nc.tensor.matmul(ps[:], o1s[:], bvs[:], start=False, stop=True)
            nc.scalar.activation(
                vt[i][:, :, 0:64],
                ps[:].rearrange("p (a b) -> p a b", a=8), AF.Copy)

        def a_units(t):
            us = []
            for m in range(4):
                us.append(lambda m=m: unit_k(t, m))
            for m in range(4):
                us.append(lambda m=m: unit_q(t, m))
            for s_ in range(4):
                us.append(lambda s_=s_: unit_v(t, s_))
            return us

        # ---- partial out projection for row chunk c of sq tile j ----
        def emit_outproj(j, c):
            at_tiles = [at_map[(j, p)] for p in range(4)]
            po = pop.tile([128, HID], BF16, tag="po")
            for o in range(2):
                ps = pp.tile([128, SQT], F32, tag="pq")
                for dch in range(4):
                    nc.tensor.matmul(
                        ps[:], at_tiles[dch][:, 128 * c:128 * (c + 1)],
                        wos[:, HID * dch + SQT * o:
                            HID * dch + SQT * (o + 1)],
                        start=(dch == 0), stop=(dch == 3))
                nc.vector.tensor_copy(po[:, SQT * o:SQT * (o + 1)], ps[:])
            r0 = SQT * j + 128 * c
            nc.sync.dma_start(po_d[r0:r0 + 128, :], po[:])
            if j < NSQT - 1:
                if c in (1, 3):
                    h0 = SQT * j + 256 * (c // 2)
                    k = 2 * j + c // 2
                    nc.gpsimd.collective_compute(
                        "ReduceScatter", OP.add, replica_groups=GROUPS,
                        ins=[po_d[h0:h0 + 256, :]],
                        outs=[rsd[k][:]])
            else:
                nc.gpsimd.collective_compute(
                    "ReduceScatter", OP.add, replica_groups=GROUPS,
                    ins=[po_d[r0:r0 + 128, :]],
                    outs=[rs3[c][:]])

        # ---- residual + LayerNorm for a pair of output chunks ----
        def ln_load(k):
            rs = lp.tile([128, HID], BF16, tag="rs")
            if k < 6:
                nc.sync.dma_start(rs[:], rsd[k][:])
            else:
                nc.sync.dma_start(rs[0:64, :], rs3[2 * (k - 6)][:])
                nc.sync.dma_start(rs[64:128, :], rs3[2 * (k - 6) + 1][:])
            xc = lp.tile([128, HID], F32, tag="xc")
            nc.sync.dma_start(xc[:], xh[128 * k:128 * (k + 1), :])
            y = lp.tile([128, HID], F32, tag="y")
            nc.vector.tensor_tensor(y[:], rs[:], xc[:], op=OP.add)
            st6 = lsp.tile([128, 12], F32, tag="st6")
            nc.vector.bn_stats(st6[:, 0:6], y[:, 0:512])
            nc.vector.bn_stats(st6[:, 6:12], y[:, 512:1024])
            mv = lsp.tile([128, 2], F32, tag="mv")
            nc.vector.bn_aggr(mv[:], st6[:])
            return rs, xc, y, mv

        def emit_ln_pair(k0):
            a = ln_load(k0)
            b = ln_load(k0 + 1)
            ve = lsp.tile([128, 2], F32, tag="ve")
            nc.vector.tensor_scalar_add(ve[:, 0:1], a[3][:, 1:2], epsc[:])
            nc.vector.tensor_scalar_add(ve[:, 1:2], b[3][:, 1:2], epsc[:])
            # 1/sqrt(ve) on DVE: 1/ve seed + 4 Newton iterations
            ry = lsp.tile([128, 2], F32, tag="ry")
            nc.vector.reciprocal_approx_fast(ry[:], ve[:])
            tmp = lsp.tile([128, 2], F32, tag="tmp")
            for _ in range(4):
                nc.vector.tensor_mul(tmp[:], ry[:], ry[:])
                nc.vector.tensor_mul(tmp[:], tmp[:], ve[:])
                nc.vector.tensor_scalar(tmp[:], tmp[:], -0.5, 1.5,
                                        op0=OP.mult, op1=OP.add)
                nc.vector.tensor_mul(ry[:], ry[:], tmp[:])
            for idx, (rs, xc, y, mv) in enumerate((a, b)):
                nc.vector.tensor_scalar(xc[:], y[:], mv[:, 0:1],
                                        ry[:, idx:idx + 1],
                                        op0=OP.subtract, op1=OP.mult)
                if apply_gb:
                    nc.vector.tensor_mul(xc[:], xc[:], gms[:])
                    nc.vector.tensor_add(xc[:], xc[:], bts[:])
                k = k0 + idx
                nc.sync.dma_start(out[128 * k:128 * (k + 1), :], xc[:])

        # ---- attention p-group for sq tile j ----
        def emit_attn_p(j, p):
            qt_ = qts_map[(j, p)]
            pv2 = app.tile([65, 2 * SQT], F32, tag="pv2")
            last = 4 * j + 3
            pend = None
            for i in range(4 * j + 4):
                d = i - 4 * j
                lo = 128 * d if d >= 0 else 0
                s2 = sp.tile([128, 2 * SQT], F32, tag="s2")
                nc.tensor.matmul(
                    s2[:, lo:SQT],
                    kt[p][0:64, 128 * i:128 * (i + 1)],
                    qt_[0:64, lo:SQT],
                    start=True, stop=True, tile_position=(0, 0))
                nc.tensor.matmul(
                    s2[:, SQT + lo:2 * SQT],
                    kt[p][64:128, 128 * i:128 * (i + 1)],
                    qt_[64:128, lo:SQT],
                    start=True, stop=True, tile_position=(64, 0))
                e2 = ep.tile([128, 2 * SQT], BF16, tag="e2")
                s2v = s2[:].rearrange("p (a b) -> p a b", a=2)
                e2v = e2[:].rearrange("p (a b) -> p a b", a=2)
                nc.scalar.activation(e2v[:, :, lo:SQT], s2v[:, :, lo:SQT],
                                     AF.Exp, scale=0.125)
                if d >= 0:
                    nc.vector.tensor_mul(
                        e2[:, lo:lo + 128], e2[:, lo:lo + 128], mask[:])
                    nc.vector.tensor_mul(
                        e2[:, SQT + lo:SQT + lo + 128],
                        e2[:, SQT + lo:SQT + lo + 128], mask[:])
                if pend is not None:
                    pl, pe2 = pend
                    nc.tensor.matmul(
                        pv2[:, pl:SQT], vt[i - 1][:, 2 * p, :],
                        pe2[:, pl:SQT], start=(i - 1 == 0), stop=False)
                    nc.tensor.matmul(
                        pv2[:, SQT + pl:2 * SQT], vt[i - 1][:, 2 * p + 1, :],
                        pe2[:, SQT + pl:2 * SQT],
                        start=(i - 1 == 0), stop=False)
                pend = (lo, e2)
            pl, pe2 = pend
            nc.tensor.matmul(
                pv2[:, pl:SQT], vt[last][:, 2 * p, :],
                pe2[:, pl:SQT], start=(last == 0), stop=True)
            nc.tensor.matmul(
                pv2[:, SQT + pl:2 * SQT], vt[last][:, 2 * p + 1, :],
                pe2[:, SQT + pl:2 * SQT],
                start=(last == 0), stop=True)
            sm = rp.tile([1, 2 * SQT], F32, tag="sm")
            nc.vector.tensor_copy(sm[:], pv2[64:65, :])
            rc = rp.tile([1, 2 * SQT], F32, tag="rc")
            nc.vector.reciprocal_approx_fast(rc[:], sm[:])
            rb = rbp.tile([64, 2 * SQT], F32, tag="rb")
            nc.gpsimd.partition_broadcast(rb[:], rc[:])
            at_ = atp.tile([128, SQT], BF16, tag=f"at{p}")
            nc.vector.tensor_tensor(at_[0:64, :], pv2[0:64, 0:SQT],
                                    rb[:, 0:SQT], op=OP.mult)
            nc.vector.tensor_tensor(at_[64:128, :], pv2[0:64, SQT:2 * SQT],
                                    rb[:, SQT:2 * SQT], op=OP.mult)
            at_map[(j, p)] = at_

        # ---- emission schedule ----
        for u in a_units(0):
            u()
        for j in range(NSQT):
            nxt = a_units(j + 1) if j + 1 < NSQT else []
            for p in range(4):
                emit_attn_p(j, p)
                for u in nxt[3 * p:3 * p + 3]:
                    u()
                if j >= 1:
                    if p == 0:
                        emit_outproj(j - 1, 0)
                        emit_outproj(j - 1, 1)
                    elif p == 1:
                        emit_outproj(j - 1, 2)
                        emit_outproj(j - 1, 3)
                    elif p == 2 and j >= 2:
                        emit_ln_pair(2 * (j - 2))
        emit_outproj(NSQT - 1, 0)
        emit_outproj(NSQT - 1, 1)
        emit_ln_pair(2 * (NSQT - 2))
        emit_outproj(NSQT - 1, 2)
        emit_outproj(NSQT - 1, 3)
        emit_ln_pair(2 * (NSQT - 1))

    nc.compile()
    return nc


def _prep_inputs(x, Wq, bq, Wk, bk, Wv, bv, Wo, bo, gamma, beta):
    """Shard + lay out the full inputs for the 8 cores."""
    f32 = np.float32
    x = np.asarray(x, f32)
    Wq, bq = np.asarray(Wq, f32), np.asarray(bq, f32)
    Wk, bk = np.asarray(Wk, f32), np.asarray(bk, f32)
    Wv, bv = np.asarray(Wv, f32), np.asarray(bv, f32)
    Wo, bo = np.asarray(Wo, f32), np.asarray(bo, f32)
    gamma, beta = np.asarray(gamma, f32), np.asarray(beta, f32)

    mask = np.triu(np.ones((128, 128), f32)).astype(BFNP)
    vone = np.ones((128, 8), BFNP)
    one1 = np.ones((1, 128), BFNP)
    gmb = np.ascontiguousarray(np.broadcast_to(gamma, (128, HID)))
    btb = np.ascontiguousarray(np.broadcast_to(beta, (128, HID)))

    def stage_w(WT):
        # [1024, 512] -> [128, 8*512] with col block hh = rows 128hh
        return np.ascontiguousarray(
            WT.reshape(8, 128, HW).transpose(1, 0, 2).reshape(128, 8 * HW)
        ).astype(BFNP)

    halves = []
    for h in range(2):
        sl = slice(HW * h, HW * (h + 1))
        woT = Wo[:, sl].T  # [512, 1024]
        halves.append(dict(
            wqs=stage_w(np.ascontiguousarray(Wq.T[:, sl])),
            wks=stage_w(np.ascontiguousarray(Wk.T[:, sl])),
            wvs=stage_w(np.ascontiguousarray(Wv.T[:, sl])),
            wos=np.ascontiguousarray(
                woT.reshape(4, 128, HID).transpose(1, 0, 2)
                .reshape(128, 4 * HID)).astype(BFNP),
            bq4=np.ascontiguousarray(bq[sl].reshape(4, 128).T),
            bk4=np.ascontiguousarray(bk[sl].reshape(4, 128).T),
            bv1=np.ascontiguousarray(bv[sl].reshape(1, HW)).astype(BFNP),
        ))

    def row_blocks(h):
        # output chunk k -> list of (global row start, nrows)
        blocks = []
        for k in range(6):
            blocks.append([(256 * k + 128 * h, 128)])
        blocks.append([(1536 + 64 * h, 64), (1664 + 64 * h, 64)])
        blocks.append([(1792 + 64 * h, 64), (1920 + 64 * h, 64)])
        return blocks

    in_maps = []
    for c in range(N_CORES):
        b, h = c // 2, c % 2
        m = dict(halves[h])
        xT = np.ascontiguousarray(x[b].T).astype(BFNP)  # [1024, 2048]
        # [1024, 2048] -> per tile t: [128, 8*512], col block hh = rows 128hh
        xr = xT.reshape(8, 128, NSQT, SQT)
        for t in range(NSQT):
            m[f"xst{t}"] = np.ascontiguousarray(
                xr[:, :, t, :].transpose(1, 0, 2).reshape(128, 8 * SQT))
        m["xh"] = np.ascontiguousarray(np.concatenate(
            [x[b, r0:r0 + n, :] for blk in row_blocks(h)
             for (r0, n) in blk], axis=0) + bo)
        m["gmb"] = gmb
        m["btb"] = btb
        m["m128"] = mask
        m["vone"] = vone
        m["one1"] = one1
        in_maps.append(m)
    return in_maps


def _run(inputs, trace=False):
    gamma = np.asarray(inputs["gamma"], np.float32)
    beta = np.asarray(inputs["beta"], np.float32)
    apply_gb = not (np.allclose(gamma, 1.0) and np.allclose(beta, 0.0))
    key = ("nc", apply_gb)
    if key not in _CACHE:
        _CACHE[key] = _build(apply_gb)
    nc = _CACHE[key]
    in_maps = _prep_inputs(**inputs)
    res = run_bass_kernel_spmd(nc, in_maps, list(range(N_CORES)),
                               trace=trace)
    out = np.empty((B, S, HID), np.float32)
    for c in range(N_CORES):
        b, h = c // 2, c % 2
        o = res.results[c]["out"]
        row = 0
        for k in range(6):
            out[b, 256 * k + 128 * h:256 * k + 128 * h + 128, :] = \
                o[row:row + 128, :]
            row += 128
        for r0 in (1536 + 64 * h, 1664 + 64 * h, 1792 + 64 * h,
                   1920 + 64 * h):
            out[b, r0:r0 + 64, :] = o[row:row + 64, :]
            row += 64
    return out, res


def kernel(**inputs):
    out, _ = _run(inputs, trace=False)
    return out


# revision 10
# speedup vs baseline: 1.0592x; 1.0592x over previous
"""Causal self-attention block (QKV proj + causal MHA + out proj + residual
+ LayerNorm) for B=4, S=2048, HID=1024, 16 heads, on 8 Trainium2 cores.

Sharding: core c handles batch b=c//2 and heads [8h, 8h+8) where h=c%2
(Megatron-style head split within a batch pair). Each core computes its 8
heads' attention and a partial output projection over the full 2048 rows;
the two cores of a batch pair combine partials with pairwise bf16
ReduceScatters (chunked, pipelined with compute; the final tile uses 4
finer chunks to drain the tail), then each core applies residual +
LayerNorm to its quarter-rows and returns [1024, 1024].

All matmuls run in bf16 (fp32 PSUM accumulation). Attention uses the
transposed-score layout (scoresT[sk, sq]): softmax sums fall out of the
PV matmul via an appended ones-row on V, causal structure shrinks
above-diagonal tiles, and each head pair shares fused two-bank PSUM
tiles so one ACT exp covers both heads; the score matmul for tile i+1 is
emitted ahead of PV(i) so the PE never waits on the exp. The Scalar
engine runs only Exp/Identity/Copy (single activation table, no
reloads); the LN rsqrt is computed on the Vector engine via
reciprocal seed + Newton iterations. Projection work for tile t+1 and
the out projection for tile j-1 are interleaved into attention tile j's
emission to keep the PE dense (p-state) and busy during softmax
normalization windows; LayerNorm chunks are deferred until well after
their ReduceScatter fires, use per-chunk scatter tensors (exact deps),
and all LN DMAs ride the sync queue so collective latency never blocks
the gpsimd queue feeding attention.
"""

import numpy as np
import ml_dtypes

import concourse.bacc as bacc
import concourse.mybir as mybir
import concourse.tile as tile
from concourse.bass_utils import run_bass_kernel_spmd

F32 = mybir.dt.float32
BF16 = mybir.dt.bfloat16
AF = mybir.ActivationFunctionType
OP = mybir.AluOpType
BFNP = ml_dtypes.bfloat16

N_CORES = 8
B, S, HID = 4, 2048, 1024
NHC = 8          # heads per core
DH = 64          # head dim
HW = 512         # per-core head width (NHC * DH)
SQT = 512        # sq tile width
NSQT = S // SQT  # 4
NHCH = HID // 128  # 8 hid chunks
SH = S // 2      # rows per core in the epilogue
EPS = 1e-5
GROUPS = [[0, 1], [2, 3], [4, 5], [6, 7]]

_CACHE = {}


def _build(apply_gb):
    nc = bacc.Bacc("TRN2", target_bir_lowering=False, debug=False,
                   num_devices=N_CORES)

    xst_d = [nc.dram_tensor(f"xst{t}", [128, 8 * SQT], BF16,
                            kind="ExternalInput").ap() for t in range(NSQT)]
    xh = nc.dram_tensor("xh", [SH, HID], F32, kind="ExternalInput").ap()
    wqs_d = nc.dram_tensor("wqs", [128, 8 * HW], BF16,
                           kind="ExternalInput").ap()
    wks_d = nc.dram_tensor("wks", [128, 8 * HW], BF16,
                           kind="ExternalInput").ap()
    wvs_d = nc.dram_tensor("wvs", [128, 8 * HW], BF16,
                           kind="ExternalInput").ap()
    wos_d = nc.dram_tensor("wos", [128, 4 * HID], BF16,
                           kind="ExternalInput").ap()
    bq4 = nc.dram_tensor("bq4", [128, 4], F32, kind="ExternalInput").ap()
    bk4 = nc.dram_tensor("bk4", [128, 4], F32, kind="ExternalInput").ap()
    bv1 = nc.dram_tensor("bv1", [1, HW], BF16, kind="ExternalInput").ap()
    one1 = nc.dram_tensor("one1", [1, 128], BF16, kind="ExternalInput").ap()
    vone = nc.dram_tensor("vone", [128, 8], BF16, kind="ExternalInput").ap()
    m128 = nc.dram_tensor("m128", [128, 128], BF16, kind="ExternalInput").ap()
    gmb = nc.dram_tensor("gmb", [128, HID], F32, kind="ExternalInput").ap()
    btb = nc.dram_tensor("btb", [128, HID], F32, kind="ExternalInput").ap()

    out = nc.dram_tensor("out", [SH, HID], F32, kind="ExternalOutput").ap()

    po_d = nc.dram_tensor("po_d", [S, HID], BF16).ap()
    # per-chunk scatter outputs so LayerNorm dma deps are exact
    rsd = [nc.dram_tensor(f"rs{k}", [128, HID], BF16).ap() for k in range(6)]
    rs3 = [nc.dram_tensor(f"rs3_{c}", [64, HID], BF16).ap() for c in range(4)]

    from contextlib import ExitStack
    with tile.TileContext(nc) as tc, ExitStack() as es:
        TP = tc.tile_pool
        cp = es.enter_context(TP(name="consts", bufs=1))
        xsp = es.enter_context(TP(name="xs", bufs=1))
        wp = es.enter_context(TP(name="w", bufs=1))
        ktp = es.enter_context(TP(name="kt", bufs=1))
        vtp = es.enter_context(TP(name="vt", bufs=1))
        qtp = es.enter_context(TP(name="qt", bufs=2))
        ep = es.enter_context(TP(name="exp", bufs=2))
        atp = es.enter_context(TP(name="att", bufs=2))
        pop = es.enter_context(TP(name="po", bufs=2))
        rp = es.enter_context(TP(name="rcp", bufs=2))
        rbp = es.enter_context(TP(name="rb", bufs=2))
        lp = es.enter_context(TP(name="ln", bufs=2))
        lsp = es.enter_context(TP(name="lns", bufs=2))
        pp = es.enter_context(TP(name="pp", bufs=2, space="PSUM"))
        sp = es.enter_context(TP(name="sp", bufs=2, space="PSUM"))
        app = es.enter_context(TP(name="ap", bufs=1, space="PSUM"))

        # ---- staged preload: one DMA per weight group / x tile, spread
        # across queues so issue cost doesn't serialize ----
        wqs = wp.tile([128, 8 * HW], BF16, name="wqs")
        nc.sync.dma_start(wqs[:], wqs_d[:])
        xst = [xsp.tile([128, 8 * SQT], BF16, name=f"xst{t}")
               for t in range(NSQT)]
        nc.gpsimd.dma_start(xst[0][:], xst_d[0][:])
        wks = wp.tile([128, 8 * HW], BF16, name="wks")
        nc.scalar.dma_start(wks[:], wks_d[:])
        nc.gpsimd.dma_start(xst[1][:], xst_d[1][:])
        wvs = wp.tile([128, 8 * HW], BF16, name="wvs")
        nc.sync.dma_start(wvs[:], wvs_d[:])
        nc.sync.dma_start(xst[2][:], xst_d[2][:])
        wos = wp.tile([128, 4 * HID], BF16, name="wos")
        nc.scalar.dma_start(wos[:], wos_d[:])
        nc.gpsimd.dma_start(xst[3][:], xst_d[3][:])

        # ---- constants ----
        mask = cp.tile([128, 128], BF16)
        nc.sync.dma_start(mask[:], m128[:])
        bqs = cp.tile([128, 4], F32)
        nc.sync.dma_start(bqs[:], bq4[:])
        bks = cp.tile([128, 4], F32)
        nc.sync.dma_start(bks[:], bk4[:])
        bvs = cp.tile([1, HW], BF16)
        nc.sync.dma_start(bvs[:], bv1[:])
        o1s = cp.tile([1, 128], BF16)
        nc.sync.dma_start(o1s[:], one1[:])
        vos = cp.tile([128, 8], BF16)
        nc.sync.dma_start(vos[:], vone[:])
        epsc = cp.tile([128, 1], F32)
        nc.vector.memset(epsc[:], EPS)
        if apply_gb:
            gms = cp.tile([128, HID], F32)
            nc.sync.dma_start(gms[:], gmb[:])
            bts = cp.tile([128, HID], F32)
            nc.sync.dma_start(bts[:], btb[:])

        kt = [ktp.tile([128, S], BF16, name=f"kt{p}") for p in range(4)]
        vt = [vtp.tile([128, 8, 65], BF16, name=f"vt{i}") for i in range(16)]
        for i in range(16):
            nc.vector.tensor_copy(
                vt[i][:, :, 64:65],
                vos[:].rearrange("p (a b) -> p a b", a=8))

        qts_map = {}
        at_map = {}

        def wsl(ws, hh):
            return ws[:, HW * hh:HW * (hh + 1)]

        def xsl(t, hh, c0, w):
            return xst[t][:, SQT * hh + c0:SQT * hh + c0 + w]

        # ---- phase-A units: projections for sq tile t ----
        def unit_q(t, m):
            ps = pp.tile([128, SQT], F32, tag="pq")
            for hh in range(NHCH):
                nc.tensor.matmul(
                    ps[:], wsl(wqs, hh)[:, 128 * m:128 * (m + 1)],
                    xsl(t, hh, 0, SQT),
                    start=(hh == 0), stop=(hh == NHCH - 1))
            qt_ = qtp.tile([128, SQT], BF16, tag=f"q{m}")
            nc.scalar.activation(qt_[:], ps[:], AF.Identity,
                                 bias=bqs[:, m:m + 1])
            qts_map[(t, m)] = qt_

        def unit_k(t, m):
            ps = pp.tile([128, SQT], F32, tag="pq")
            for hh in range(NHCH):
                nc.tensor.matmul(
                    ps[:], wsl(wks, hh)[:, 128 * m:128 * (m + 1)],
                    xsl(t, hh, 0, SQT),
                    start=(hh == 0), stop=(hh == NHCH - 1))
            nc.scalar.activation(kt[m][:, SQT * t:SQT * (t + 1)], ps[:],
                                 AF.Identity, bias=bks[:, m:m + 1])

        def unit_v(t, s_):
            i = 4 * t + s_
            ps = pp.tile([128, HW], F32, tag="pq")
            for hh in range(NHCH):
                nc.tensor.matmul(
                    ps[:], xsl(t, hh, 128 * s_, 128), wsl(wvs, hh),
                    start=(hh == 0), stop=False)
            nc.tensor.matmul(ps[:], o1s[:], bvs[:], start=False, stop=True)
            nc.scalar.activation(
                vt[i][:, :, 0:64],
                ps[:].rearrange("p (a b) -> p a b", a=8), AF.Copy)

        def a_units(t):
            us = []
            for m in range(4):
                us.append(lambda m=m: unit_k(t, m))
            for m in range(4):
                us.append(lambda m=m: unit_q(t, m))
            for s_ in range(4):
                us.append(lambda s_=s_: unit_v(t, s_))
            return us

        # ---- partial out projection for row chunk c of sq tile j ----
        def emit_outproj(j, c):
            at_tiles = [at_map[(j, p)] for p in range(4)]
            po = pop.tile([128, HID], BF16, tag="po")
            for o in range(2):
                ps = pp.tile([128, SQT], F32, tag="pq")
                for dch in range(4):
                    nc.tensor.matmul(
                        ps[:], at_tiles[dch][:, 128 * c:128 * (c + 1)],
                        wos[:, HID * dch + SQT * o:
                            HID * dch + SQT * (o + 1)],
                        start=(dch == 0), stop=(dch == 3))
                nc.scalar.activation(po[:, SQT * o:SQT * (o + 1)], ps[:],
                                     AF.Copy)
            r0 = SQT * j + 128 * c
            nc.sync.dma_start(po_d[r0:r0 + 128, :], po[:])
            if j < NSQT - 1:
                if c in (1, 3):
                    h0 = SQT * j + 256 * (c // 2)
                    k = 2 * j + c // 2
                    nc.gpsimd.collective_compute(
                        "ReduceScatter", OP.add, replica_groups=GROUPS,
                        ins=[po_d[h0:h0 + 256, :]],
                        outs=[rsd[k][:]])
            else:
                nc.gpsimd.collective_compute(
                    "ReduceScatter", OP.add, replica_groups=GROUPS,
                    ins=[po_d[r0:r0 + 128, :]],
                    outs=[rs3[c][:]])

        # ---- residual + LayerNorm for a pair of output chunks ----
        def ln_load(k):
            rs = lp.tile([128, HID], BF16, tag="rs")
            if k < 6:
                nc.sync.dma_start(rs[:], rsd[k][:])
            else:
                nc.sync.dma_start(rs[0:64, :], rs3[2 * (k - 6)][:])
                nc.sync.dma_start(rs[64:128, :], rs3[2 * (k - 6) + 1][:])
            xc = lp.tile([128, HID], F32, tag="xc")
            nc.sync.dma_start(xc[:], xh[128 * k:128 * (k + 1), :])
            y = lp.tile([128, HID], F32, tag="y")
            nc.vector.tensor_tensor(y[:], rs[:], xc[:], op=OP.add)
            st6 = lsp.tile([128, 12], F32, tag="st6")
            nc.vector.bn_stats(st6[:, 0:6], y[:, 0:512])
            nc.vector.bn_stats(st6[:, 6:12], y[:, 512:1024])
            mv = lsp.tile([128, 2], F32, tag="mv")
            nc.vector.bn_aggr(mv[:], st6[:])
            return rs, xc, y, mv

        def emit_ln_pair(k0):
            a = ln_load(k0)
            b = ln_load(k0 + 1)
            ve = lsp.tile([128, 2], F32, tag="ve")
            nc.vector.tensor_scalar_add(ve[:, 0:1], a[3][:, 1:2], epsc[:])
            nc.vector.tensor_scalar_add(ve[:, 1:2], b[3][:, 1:2], epsc[:])
            # 1/sqrt(ve) on DVE: 1/ve seed + 4 Newton iterations
            ry = lsp.tile([128, 2], F32, tag="ry")
            nc.vector.reciprocal_approx_fast(ry[:], ve[:])
            tmp = lsp.tile([128, 2], F32, tag="tmp")
            for _ in range(4):
                nc.vector.tensor_mul(tmp[:], ry[:], ry[:])
                nc.vector.tensor_mul(tmp[:], tmp[:], ve[:])
                nc.vector.tensor_scalar(tmp[:], tmp[:], -0.5, 1.5,
                                        op0=OP.mult, op1=OP.add)
                nc.vector.tensor_mul(ry[:], ry[:], tmp[:])
            for idx, (rs, xc, y, mv) in enumerate((a, b)):
                nc.vector.tensor_scalar(xc[:], y[:], mv[:, 0:1],
                                        ry[:, idx:idx + 1],
                                        op0=OP.subtract, op1=OP.mult)
                if apply_gb:
                    nc.vector.tensor_mul(xc[:], xc[:], gms[:])
                    nc.vector.tensor_add(xc[:], xc[:], bts[:])
                k = k0 + idx
                nc.sync.dma_start(out[128 * k:128 * (k + 1), :], xc[:])

        # ---- attention p-group for sq tile j ----
        def emit_attn_p(j, p):
            qt_ = qts_map[(j, p)]
            pv2 = app.tile([65, 2 * SQT], F32, tag="pv2")
            last = 4 * j + 3
            pend = None
            for i in range(4 * j + 4):
                d = i - 4 * j
                lo = 128 * d if d >= 0 else 0
                s2 = sp.tile([128, 2 * SQT], F32, tag="s2")
                nc.tensor.matmul(
                    s2[:, lo:SQT],
                    kt[p][0:64, 128 * i:128 * (i + 1)],
                    qt_[0:64, lo:SQT],
                    start=True, stop=True, tile_position=(0, 0))
                nc.tensor.matmul(
                    s2[:, SQT + lo:2 * SQT],
                    kt[p][64:128, 128 * i:128 * (i + 1)],
                    qt_[64:128, lo:SQT],
                    start=True, stop=True, tile_position=(64, 0))
                e2 = ep.tile([128, 2 * SQT], BF16, tag="e2")
                s2v = s2[:].rearrange("p (a b) -> p a b", a=2)
                e2v = e2[:].rearrange("p (a b) -> p a b", a=2)
                nc.scalar.activation(e2v[:, :, lo:SQT], s2v[:, :, lo:SQT],
                                     AF.Exp, scale=0.125)
                if d >= 0:
                    nc.vector.tensor_mul(
                        e2[:, lo:lo + 128], e2[:, lo:lo + 128], mask[:])
                    nc.vector.tensor_mul(
                        e2[:, SQT + lo:SQT + lo + 128],
                        e2[:, SQT + lo:SQT + lo + 128], mask[:])
                if pend is not None:
                    pl, pe2 = pend
                    nc.tensor.matmul(
                        pv2[:, pl:SQT], vt[i - 1][:, 2 * p, :],
                        pe2[:, pl:SQT], start=(i - 1 == 0), stop=False)
                    nc.tensor.matmul(
                        pv2[:, SQT + pl:2 * SQT], vt[i - 1][:, 2 * p + 1, :],
                        pe2[:, SQT + pl:2 * SQT],
                        start=(i - 1 == 0), stop=False)
                pend = (lo, e2)
            pl, pe2 = pend
            nc.tensor.matmul(
                pv2[:, pl:SQT], vt[last][:, 2 * p, :],
                pe2[:, pl:SQT], start=(last == 0), stop=True)
            nc.tensor.matmul(
                pv2[:, SQT + pl:2 * SQT], vt[last][:, 2 * p + 1, :],
                pe2[:, SQT + pl:2 * SQT],
                start=(last == 0), stop=True)
            sm = rp.tile([1, 2 * SQT], F32, tag="sm")
            nc.vector.tensor_copy(sm[:], pv2[64:65, :])
            rc = rp.tile([1, 2 * SQT], F32, tag="rc")
            nc.vector.reciprocal_approx_fast(rc[:], sm[:])
            rb = rbp.tile([64, 2 * SQT], F32, tag="rb")
            nc.gpsimd.partition_broadcast(rb[:], rc[:])
            at_ = atp.tile([128, SQT], BF16, tag=f"at{p}")
            nc.vector.tensor_tensor(at_[0:64, :], pv2[0:64, 0:SQT],
                                    rb[:, 0:SQT], op=OP.mult)
            nc.vector.tensor_tensor(at_[64:128, :], pv2[0:64, SQT:2 * SQT],
                                    rb[:, SQT:2 * SQT], op=OP.mult)
            at_map[(j, p)] = at_

        # ---- emission schedule ----
        for u in a_units(0):
            u()
        for j in range(NSQT):
            nxt = a_units(j + 1) if j + 1 < NSQT else []
            for p in range(4):
                emit_attn_p(j, p)
                for u in nxt[3 * p:3 * p + 3]:
                    u()
                if p == 1 and j >= 2:
                    emit_ln_pair(2 * (j - 2))
                if j >= 1:
                    if p == 2:
                        emit_outproj(j - 1, 0)
                        emit_outproj(j - 1, 1)
                    elif p == 3:
                        emit_outproj(j - 1, 2)
                        emit_outproj(j - 1, 3)
        emit_outproj(NSQT - 1, 0)
        emit_outproj(NSQT - 1, 1)
        emit_ln_pair(2 * (NSQT - 2))
        emit_outproj(NSQT - 1, 2)
        emit_outproj(NSQT - 1, 3)
        emit_ln_pair(2 * (NSQT - 1))

    nc.compile()
    return nc


def _prep_inputs(x, Wq, bq, Wk, bk, Wv, bv, Wo, bo, gamma, beta):
    """Shard + lay out the full inputs for the 8 cores."""
    f32 = np.float32
    x = np.asarray(x, f32)
    Wq, bq = np.asarray(Wq, f32), np.asarray(bq, f32)
    Wk, bk = np.asarray(Wk, f32), np.asarray(bk, f32)
    Wv, bv = np.asarray(Wv, f32), np.asarray(bv, f32)
    Wo, bo = np.asarray(Wo, f32), np.asarray(bo, f32)
    gamma, beta = np.asarray(gamma, f32), np.asarray(beta, f32)

    mask = np.triu(np.ones((128, 128), f32)).astype(BFNP)
    vone = np.ones((128, 8), BFNP)
    one1 = np.ones((1, 128), BFNP)
    gmb = np.ascontiguousarray(np.broadcast_to(gamma, (128, HID)))
    btb = np.ascontiguousarray(np.broadcast_to(beta, (128, HID)))

    def stage_w(WT):
        # [1024, 512] -> [128, 8*512] with col block hh = rows 128hh
        return np.ascontiguousarray(
            WT.reshape(8, 128, HW).transpose(1, 0, 2).reshape(128, 8 * HW)
        ).astype(BFNP)

    halves = []
    for h in range(2):
        sl = slice(HW * h, HW * (h + 1))
        woT = Wo[:, sl].T  # [512, 1024]
        halves.append(dict(
            wqs=stage_w(np.ascontiguousarray(Wq.T[:, sl])),
            wks=stage_w(np.ascontiguousarray(Wk.T[:, sl])),
            wvs=stage_w(np.ascontiguousarray(Wv.T[:, sl])),
            wos=np.ascontiguousarray(
                woT.reshape(4, 128, HID).transpose(1, 0, 2)
                .reshape(128, 4 * HID)).astype(BFNP),
            bq4=np.ascontiguousarray(bq[sl].reshape(4, 128).T),
            bk4=np.ascontiguousarray(bk[sl].reshape(4, 128).T),
            bv1=np.ascontiguousarray(bv[sl].reshape(1, HW)).astype(BFNP),
        ))

    def row_blocks(h):
        # output chunk k -> list of (global row start, nrows)
        blocks = []
        for k in range(6):
            blocks.append([(256 * k + 128 * h, 128)])
        blocks.append([(1536 + 64 * h, 64), (1664 + 64 * h, 64)])
        blocks.append([(1792 + 64 * h, 64), (1920 + 64 * h, 64)])
        return blocks

    in_maps = []
    for c in range(N_CORES):
        b, h = c // 2, c % 2
        m = dict(halves[h])
        xT = np.ascontiguousarray(x[b].T).astype(BFNP)  # [1024, 2048]
        # [1024, 2048] -> per tile t: [128, 8*512], col block hh = rows 128hh
        xr = xT.reshape(8, 128, NSQT, SQT)
        for t in range(NSQT):
            m[f"xst{t}"] = np.ascontiguousarray(
                xr[:, :, t, :].transpose(1, 0, 2).reshape(128, 8 * SQT))
        m["xh"] = np.ascontiguousarray(np.concatenate(
            [x[b, r0:r0 + n, :] for blk in row_blocks(h)
             for (r0, n) in blk], axis=0) + bo)
        m["gmb"] = gmb
        m["btb"] = btb
        m["m128"] = mask
        m["vone"] = vone
        m["one1"] = one1
        in_maps.append(m)
    return in_maps


def _run(inputs, trace=False):
    gamma = np.asarray(inputs["gamma"], np.float32)
    beta = np.asarray(inputs["beta"], np.float32)
    apply_gb = not (np.allclose(gamma, 1.0) and np.allclose(beta, 0.0))
    key = ("nc", apply_gb)
    if key not in _CACHE:
        _CACHE[key] = _build(apply_gb)
    nc = _CACHE[key]
    in_maps = _prep_inputs(**inputs)
    res = run_bass_kernel_spmd(nc, in_maps, list(range(N_CORES)),
                               trace=trace)
    out = np.empty((B, S, HID), np.float32)
    for c in range(N_CORES):
        b, h = c // 2, c % 2
        o = res.results[c]["out"]
        row = 0
        for k in range(6):
            out[b, 256 * k + 128 * h:256 * k + 128 * h + 128, :] = \
                o[row:row + 128, :]
            row += 128
        for r0 in (1536 + 64 * h, 1664 + 64 * h, 1792 + 64 * h,
                   1920 + 64 * h):
            out[b, r0:r0 + 64, :] = o[row:row + 64, :]
            row += 64
    return out, res


def kernel(**inputs):
    out, _ = _run(inputs, trace=False)
    return out


# revision 11
# speedup vs baseline: 1.1032x; 1.0415x over previous
"""Causal self-attention block (QKV proj + causal MHA + out proj + residual
+ LayerNorm) for B=4, S=2048, HID=1024, 16 heads, on 8 Trainium2 cores.

Sharding: core c handles batch b=c//2 and heads [8h, 8h+8) where h=c%2
(Megatron-style head split within a batch pair). Each core computes its 8
heads' attention and a partial output projection over the full 2048 rows;
the two cores of a batch pair combine partials with pairwise bf16
ReduceScatters (chunked, pipelined with compute; the final tile uses 4
finer chunks to drain the tail), then each core applies residual +
LayerNorm to its quarter-rows and returns [1024, 1024].

All matmuls run in bf16 (fp32 PSUM accumulation). Attention uses the
transposed-score layout (scoresT[sk, sq]): softmax sums fall out of the
PV matmul via an appended ones-row on V, causal structure shrinks
above-diagonal tiles, and each head pair shares fused two-bank PSUM
tiles so one ACT exp covers both heads; the score matmul for tile i+1 is
emitted ahead of PV(i) so the PE never waits on the exp. The Scalar
engine runs only Exp/Identity/Copy (single activation table, no
reloads); the LN rsqrt is computed on the Vector engine via
reciprocal seed + Newton iterations. Projection work for tile t+1 and
the out projection for tile j-1 are interleaved into attention tile j's
emission to keep the PE dense (p-state) and busy during softmax
normalization windows; LayerNorm chunks are deferred until well after
their ReduceScatter fires, use per-chunk scatter tensors (exact deps),
and all LN DMAs ride the sync queue so collective latency never blocks
the gpsimd queue feeding attention.
"""

import numpy as np
import ml_dtypes

import concourse.bacc as bacc
import concourse.mybir as mybir
import concourse.tile as tile
from concourse.bass_utils import run_bass_kernel_spmd

F32 = mybir.dt.float32
BF16 = mybir.dt.bfloat16
AF = mybir.ActivationFunctionType
OP = mybir.AluOpType
BFNP = ml_dtypes.bfloat16
F8 = mybir.dt.float8e4
F8NP = ml_dtypes.float8_e4m3
PM = mybir.MatmulPerfMode

N_CORES = 8
B, S, HID = 4, 2048, 1024
NHC = 8          # heads per core
DH = 64          # head dim
HW = 512         # per-core head width (NHC * DH)
SQT = 512        # sq tile width
NSQT = S // SQT  # 4
NHCH = HID // 128  # 8 hid chunks
SH = S // 2      # rows per core in the epilogue
EPS = 1e-5
GROUPS = [[0, 1], [2, 3], [4, 5], [6, 7]]

_CACHE = {}


def _build(apply_gb):
    nc = bacc.Bacc("TRN2", target_bir_lowering=False, debug=False,
                   num_devices=N_CORES)

    xst_d = [nc.dram_tensor(f"xst{t}", [128, 8 * SQT], F8,
                            kind="ExternalInput").ap() for t in range(NSQT)]
    xh = nc.dram_tensor("xh", [SH, HID], F32, kind="ExternalInput").ap()
    wqs_d = nc.dram_tensor("wqs", [128, 8 * HW], F8,
                           kind="ExternalInput").ap()
    wks_d = nc.dram_tensor("wks", [128, 8 * HW], F8,
                           kind="ExternalInput").ap()
    wvs_d = nc.dram_tensor("wvs", [128, 8 * HW], F8,
                           kind="ExternalInput").ap()
    wos_d = nc.dram_tensor("wos", [128, 4 * HID], BF16,
                           kind="ExternalInput").ap()
    bq4 = nc.dram_tensor("bq4", [128, 4], F32, kind="ExternalInput").ap()
    bk4 = nc.dram_tensor("bk4", [128, 4], F32, kind="ExternalInput").ap()
    bv1 = nc.dram_tensor("bv1", [1, HW], BF16, kind="ExternalInput").ap()
    one1 = nc.dram_tensor("one1", [1, 128], BF16, kind="ExternalInput").ap()
    vone = nc.dram_tensor("vone", [128, 8], BF16, kind="ExternalInput").ap()
    m128 = nc.dram_tensor("m128", [128, 128], BF16, kind="ExternalInput").ap()
    gmb = nc.dram_tensor("gmb", [128, HID], F32, kind="ExternalInput").ap()
    btb = nc.dram_tensor("btb", [128, HID], F32, kind="ExternalInput").ap()

    out = nc.dram_tensor("out", [SH, HID], F32, kind="ExternalOutput").ap()

    po_d = nc.dram_tensor("po_d", [S, HID], BF16).ap()
    # per-chunk scatter outputs so LayerNorm dma deps are exact
    rsd = [nc.dram_tensor(f"rs{k}", [128, HID], BF16).ap() for k in range(8)]

    from contextlib import ExitStack
    with tile.TileContext(nc) as tc, ExitStack() as es:
        TP = tc.tile_pool
        cp = es.enter_context(TP(name="consts", bufs=1))
        xsp = es.enter_context(TP(name="xs", bufs=1))
        wp = es.enter_context(TP(name="w", bufs=1))
        ktp = es.enter_context(TP(name="kt", bufs=1))
        vtp = es.enter_context(TP(name="vt", bufs=1))
        qtp = es.enter_context(TP(name="qt", bufs=2))
        ep = es.enter_context(TP(name="exp", bufs=2))
        atp = es.enter_context(TP(name="att", bufs=2))
        pop = es.enter_context(TP(name="po", bufs=2))
        rp = es.enter_context(TP(name="rcp", bufs=2))
        rbp = es.enter_context(TP(name="rb", bufs=2))
        lp = es.enter_context(TP(name="ln", bufs=2))
        lsp = es.enter_context(TP(name="lns", bufs=2))
        pp = es.enter_context(TP(name="pp", bufs=2, space="PSUM"))
        sp = es.enter_context(TP(name="sp", bufs=2, space="PSUM"))
        app = es.enter_context(TP(name="ap", bufs=1, space="PSUM"))

        # ---- staged preload: one DMA per weight group / x tile, spread
        # across queues so issue cost doesn't serialize ----
        wqs = wp.tile([128, 8 * HW], F8, name="wqs")
        nc.sync.dma_start(wqs[:], wqs_d[:])
        xst = [xsp.tile([128, 8 * SQT], F8, name=f"xst{t}")
               for t in range(NSQT)]
        nc.gpsimd.dma_start(xst[0][:], xst_d[0][:])
        wks = wp.tile([128, 8 * HW], F8, name="wks")
        nc.scalar.dma_start(wks[:], wks_d[:])
        nc.gpsimd.dma_start(xst[1][:], xst_d[1][:])
        wvs = wp.tile([128, 8 * HW], F8, name="wvs")
        nc.sync.dma_start(wvs[:], wvs_d[:])
        nc.sync.dma_start(xst[2][:], xst_d[2][:])
        wos = wp.tile([128, 4 * HID], BF16, name="wos")
        nc.scalar.dma_start(wos[:], wos_d[:])
        nc.gpsimd.dma_start(xst[3][:], xst_d[3][:])

        # ---- constants ----
        mask = cp.tile([128, 128], BF16)
        nc.sync.dma_start(mask[:], m128[:])
        bqs = cp.tile([128, 4], F32)
        nc.sync.dma_start(bqs[:], bq4[:])
        bks = cp.tile([128, 4], F32)
        nc.sync.dma_start(bks[:], bk4[:])
        bvs = cp.tile([1, HW], BF16)
        nc.sync.dma_start(bvs[:], bv1[:])
        o1s = cp.tile([1, 128], BF16)
        nc.sync.dma_start(o1s[:], one1[:])
        vos = cp.tile([128, 8], BF16)
        nc.sync.dma_start(vos[:], vone[:])
        epsc = cp.tile([128, 1], F32)
        nc.vector.memset(epsc[:], EPS)
        if apply_gb:
            gms = cp.tile([128, HID], F32)
            nc.sync.dma_start(gms[:], gmb[:])
            bts = cp.tile([128, HID], F32)
            nc.sync.dma_start(bts[:], btb[:])

        kt = [ktp.tile([128, S], BF16, name=f"kt{p}") for p in range(4)]
        vt = [vtp.tile([128, 8, 65], BF16, name=f"vt{i}") for i in range(16)]
        for i in range(16):
            nc.vector.tensor_copy(
                vt[i][:, :, 64:65],
                vos[:].rearrange("p (a b) -> p a b", a=8))

        qts_map = {}
        at_map = {}

        def wpair(ws, P):
            return ws[:].rearrange("p (a b) -> p a b", a=8)[:, 2 * P:2 * P + 2,
                                                           :]

        def xpair(t, P, c0, w):
            return xst[t][:].rearrange(
                "p (a b) -> p a b", a=8)[:, 2 * P:2 * P + 2, c0:c0 + w]

        # ---- phase-A units: projections for sq tile t ----
        def unit_q(t, m):
            ps = pp.tile([128, SQT], F32, tag="pq")
            for P in range(4):
                nc.tensor.matmul(
                    ps[:], wpair(wqs, P)[:, :, 128 * m:128 * (m + 1)],
                    xpair(t, P, 0, SQT),
                    start=(P == 0), stop=(P == 3), perf_mode=PM.DoubleRow)
            qt_ = qtp.tile([128, SQT], BF16, tag=f"q{m}")
            nc.scalar.activation(qt_[:], ps[:], AF.Identity,
                                 bias=bqs[:, m:m + 1])
            qts_map[(t, m)] = qt_

        def unit_k(t, m):
            ps = pp.tile([128, SQT], F32, tag="pq")
            for P in range(4):
                nc.tensor.matmul(
                    ps[:], wpair(wks, P)[:, :, 128 * m:128 * (m + 1)],
                    xpair(t, P, 0, SQT),
                    start=(P == 0), stop=(P == 3), perf_mode=PM.DoubleRow)
            nc.scalar.activation(kt[m][:, SQT * t:SQT * (t + 1)], ps[:],
                                 AF.Identity, bias=bks[:, m:m + 1])

        def unit_v(t, s_):
            i = 4 * t + s_
            ps = pp.tile([128, HW], F32, tag="pq")
            for P in range(4):
                nc.tensor.matmul(
                    ps[:], xpair(t, P, 128 * s_, 128), wpair(wvs, P),
                    start=(P == 0), stop=False, perf_mode=PM.DoubleRow)
            nc.tensor.matmul(ps[:], o1s[:], bvs[:], start=False, stop=True)
            nc.scalar.activation(
                vt[i][:, :, 0:64],
                ps[:].rearrange("p (a b) -> p a b", a=8), AF.Copy)

        def a_units(t):
            us = []
            for m in range(4):
                us.append(lambda m=m: unit_k(t, m))
            for m in range(4):
                us.append(lambda m=m: unit_q(t, m))
            for s_ in range(4):
                us.append(lambda s_=s_: unit_v(t, s_))
            return us

        # ---- partial out projection for row chunk c of sq tile j ----
        def emit_outproj(j, c):
            at_tiles = [at_map[(j, p)] for p in range(4)]
            po = pop.tile([128, HID], BF16, tag="po")
            for o in range(2):
                ps = pp.tile([128, SQT], F32, tag="pq")
                for dch in range(4):
                    nc.tensor.matmul(
                        ps[:], at_tiles[dch][:, 128 * c:128 * (c + 1)],
                        wos[:, HID * dch + SQT * o:
                            HID * dch + SQT * (o + 1)],
                        start=(dch == 0), stop=(dch == 3))
                nc.scalar.activation(po[:, SQT * o:SQT * (o + 1)], ps[:],
                                     AF.Copy)
            r0 = SQT * j + 128 * c
            nc.sync.dma_start(po_d[r0:r0 + 128, :], po[:])
            if c in (1, 3):
                h0 = SQT * j + 256 * (c // 2)
                k = 2 * j + c // 2
                nc.gpsimd.collective_compute(
                    "ReduceScatter", OP.add, replica_groups=GROUPS,
                    ins=[po_d[h0:h0 + 256, :]],
                    outs=[rsd[k][:]])

        # ---- residual + LayerNorm for a pair of output chunks ----
        def ln_load(k):
            rs = lp.tile([128, HID], BF16, tag="rs")
            nc.sync.dma_start(rs[:], rsd[k][:])
            xc = lp.tile([128, HID], F32, tag="xc")
            nc.sync.dma_start(xc[:], xh[128 * k:128 * (k + 1), :])
            y = lp.tile([128, HID], F32, tag="y")
            nc.vector.tensor_tensor(y[:], rs[:], xc[:], op=OP.add)
            st6 = lsp.tile([128, 12], F32, tag="st6")
            nc.vector.bn_stats(st6[:, 0:6], y[:, 0:512])
            nc.vector.bn_stats(st6[:, 6:12], y[:, 512:1024])
            mv = lsp.tile([128, 2], F32, tag="mv")
            nc.vector.bn_aggr(mv[:], st6[:])
            return rs, xc, y, mv

        def emit_ln_pair(k0):
            a = ln_load(k0)
            b = ln_load(k0 + 1)
            ve = lsp.tile([128, 2], F32, tag="ve")
            nc.vector.tensor_scalar_add(ve[:, 0:1], a[3][:, 1:2], epsc[:])
            nc.vector.tensor_scalar_add(ve[:, 1:2], b[3][:, 1:2], epsc[:])
            # 1/sqrt(ve) on DVE: 1/ve seed + 4 Newton iterations
            ry = lsp.tile([128, 2], F32, tag="ry")
            nc.vector.reciprocal_approx_fast(ry[:], ve[:])
            tmp = lsp.tile([128, 2], F32, tag="tmp")
            for _ in range(4):
                nc.vector.tensor_mul(tmp[:], ry[:], ry[:])
                nc.vector.tensor_mul(tmp[:], tmp[:], ve[:])
                nc.vector.tensor_scalar(tmp[:], tmp[:], -0.5, 1.5,
                                        op0=OP.mult, op1=OP.add)
                nc.vector.tensor_mul(ry[:], ry[:], tmp[:])
            for idx, (rs, xc, y, mv) in enumerate((a, b)):
                nc.vector.tensor_scalar(xc[:], y[:], mv[:, 0:1],
                                        ry[:, idx:idx + 1],
                                        op0=OP.subtract, op1=OP.mult)
                if apply_gb:
                    nc.vector.tensor_mul(xc[:], xc[:], gms[:])
                    nc.vector.tensor_add(xc[:], xc[:], bts[:])
                k = k0 + idx
                nc.sync.dma_start(out[128 * k:128 * (k + 1), :], xc[:])

        # ---- attention p-group for sq tile j ----
        def emit_attn_p(j, p):
            qt_ = qts_map[(j, p)]
            pv2 = app.tile([65, 2 * SQT], F32, tag="pv2")
            last = 4 * j + 3
            pend = None
            for i in range(4 * j + 4):
                d = i - 4 * j
                lo = 128 * d if d >= 0 else 0
                s2 = sp.tile([128, 2 * SQT], F32, tag="s2")
                nc.tensor.matmul(
                    s2[:, lo:SQT],
                    kt[p][0:64, 128 * i:128 * (i + 1)],
                    qt_[0:64, lo:SQT],
                    start=True, stop=True, tile_position=(0, 0))
                nc.tensor.matmul(
                    s2[:, SQT + lo:2 * SQT],
                    kt[p][64:128, 128 * i:128 * (i + 1)],
                    qt_[64:128, lo:SQT],
                    start=True, stop=True, tile_position=(64, 0))
                e2 = ep.tile([128, 2 * SQT], BF16, tag="e2")
                s2v = s2[:].rearrange("p (a b) -> p a b", a=2)
                e2v = e2[:].rearrange("p (a b) -> p a b", a=2)
                nc.scalar.activation(e2v[:, :, lo:SQT], s2v[:, :, lo:SQT],
                                     AF.Exp, scale=0.125 / 256.0)
                if d >= 0:
                    nc.vector.tensor_mul(
                        e2[:, lo:lo + 128], e2[:, lo:lo + 128], mask[:])
                    nc.vector.tensor_mul(
                        e2[:, SQT + lo:SQT + lo + 128],
                        e2[:, SQT + lo:SQT + lo + 128], mask[:])
                if pend is not None:
                    pl, pe2 = pend
                    nc.tensor.matmul(
                        pv2[:, pl:SQT], vt[i - 1][:, 2 * p, :],
                        pe2[:, pl:SQT], start=(i - 1 == 0), stop=False)
                    nc.tensor.matmul(
                        pv2[:, SQT + pl:2 * SQT], vt[i - 1][:, 2 * p + 1, :],
                        pe2[:, SQT + pl:2 * SQT],
                        start=(i - 1 == 0), stop=False)
                pend = (lo, e2)
            pl, pe2 = pend
            nc.tensor.matmul(
                pv2[:, pl:SQT], vt[last][:, 2 * p, :],
                pe2[:, pl:SQT], start=(last == 0), stop=True)
            nc.tensor.matmul(
                pv2[:, SQT + pl:2 * SQT], vt[last][:, 2 * p + 1, :],
                pe2[:, SQT + pl:2 * SQT],
                start=(last == 0), stop=True)
            sm = rp.tile([1, 2 * SQT], F32, tag="sm")
            nc.vector.tensor_copy(sm[:], pv2[64:65, :])
            rc = rp.tile([1, 2 * SQT], F32, tag="rc")
            nc.vector.reciprocal_approx_fast(rc[:], sm[:])
            rb = rbp.tile([64, 2 * SQT], F32, tag="rb")
            nc.gpsimd.partition_broadcast(rb[:], rc[:])
            at_ = atp.tile([128, SQT], BF16, tag=f"at{p}")
            nc.vector.tensor_tensor(at_[0:64, :], pv2[0:64, 0:SQT],
                                    rb[:, 0:SQT], op=OP.mult)
            nc.vector.tensor_tensor(at_[64:128, :], pv2[0:64, SQT:2 * SQT],
                                    rb[:, SQT:2 * SQT], op=OP.mult)
            at_map[(j, p)] = at_

        # ---- emission schedule ----
        for u in a_units(0):
            u()
        for j in range(NSQT):
            nxt = a_units(j + 1) if j + 1 < NSQT else []
            for p in range(4):
                emit_attn_p(j, p)
                for u in nxt[3 * p:3 * p + 3]:
                    u()
                if p == 1 and j >= 2:
                    emit_ln_pair(2 * (j - 2))
                if j >= 1:
                    if p == 2:
                        emit_outproj(j - 1, 0)
                        emit_outproj(j - 1, 1)
                    elif p == 3:
                        emit_outproj(j - 1, 2)
                        emit_outproj(j - 1, 3)
        emit_outproj(NSQT - 1, 0)
        emit_outproj(NSQT - 1, 1)
        emit_ln_pair(2 * (NSQT - 2))
        emit_outproj(NSQT - 1, 2)
        emit_outproj(NSQT - 1, 3)
        emit_ln_pair(2 * (NSQT - 1))


    nc.compile()
    return nc


def _prep_inputs(x, Wq, bq, Wk, bk, Wv, bv, Wo, bo, gamma, beta):
    """Shard + lay out the full inputs for the 8 cores."""
    f32 = np.float32
    x = np.asarray(x, f32)
    Wq, bq = np.asarray(Wq, f32), np.asarray(bq, f32)
    Wk, bk = np.asarray(Wk, f32), np.asarray(bk, f32)
    Wv, bv = np.asarray(Wv, f32), np.asarray(bv, f32)
    Wo, bo = np.asarray(Wo, f32), np.asarray(bo, f32)
    gamma, beta = np.asarray(gamma, f32), np.asarray(beta, f32)

    mask = np.triu(np.ones((128, 128), f32)).astype(BFNP)
    vone = np.full((128, 8), 16.0, BFNP)
    one1 = np.ones((1, 128), BFNP)
    gmb = np.ascontiguousarray(np.broadcast_to(gamma, (128, HID)))
    btb = np.ascontiguousarray(np.broadcast_to(beta, (128, HID)))

    def stage_w(WT):
        # [1024, 512] -> [128, 8*512] with col block hh = rows 128hh,
        # scaled by 16 into the fp8 e4m3 sweet spot
        return np.ascontiguousarray(
            (WT * 16.0).reshape(8, 128, HW).transpose(1, 0, 2)
            .reshape(128, 8 * HW)).astype(F8NP)

    halves = []
    for h in range(2):
        sl = slice(HW * h, HW * (h + 1))
        woT = Wo[:, sl].T  # [512, 1024]
        halves.append(dict(
            wqs=stage_w(np.ascontiguousarray(Wq.T[:, sl])),
            wks=stage_w(np.ascontiguousarray(Wk.T[:, sl])),
            wvs=stage_w(np.ascontiguousarray(Wv.T[:, sl])),
            wos=np.ascontiguousarray(
                woT.reshape(4, 128, HID).transpose(1, 0, 2)
                .reshape(128, 4 * HID)).astype(BFNP),
            bq4=np.ascontiguousarray(16.0 * bq[sl].reshape(4, 128).T),
            bk4=np.ascontiguousarray(16.0 * bk[sl].reshape(4, 128).T),
            bv1=np.ascontiguousarray(
                16.0 * bv[sl].reshape(1, HW)).astype(BFNP),
        ))

    def row_blocks(h):
        # output chunk k -> list of (global row start, nrows)
        return [[(256 * k + 128 * h, 128)] for k in range(8)]

    in_maps = []
    for c in range(N_CORES):
        b, h = c // 2, c % 2
        m = dict(halves[h])
        xT = np.ascontiguousarray(x[b].T).astype(F8NP)  # [1024, 2048]
        # [1024, 2048] -> per tile t: [128, 8*512], col block hh = rows 128hh
        xr = xT.reshape(8, 128, NSQT, SQT)
        for t in range(NSQT):
            m[f"xst{t}"] = np.ascontiguousarray(
                xr[:, :, t, :].transpose(1, 0, 2).reshape(128, 8 * SQT))
        m["xh"] = np.ascontiguousarray(np.concatenate(
            [x[b, r0:r0 + n, :] for blk in row_blocks(h)
             for (r0, n) in blk], axis=0) + bo)
        m["gmb"] = gmb
        m["btb"] = btb
        m["m128"] = mask
        m["vone"] = vone
        m["one1"] = one1
        in_maps.append(m)
    return in_maps


def _run(inputs, trace=False):
    gamma = np.asarray(inputs["gamma"], np.float32)
    beta = np.asarray(inputs["beta"], np.float32)
    apply_gb = not (np.allclose(gamma, 1.0) and np.allclose(beta, 0.0))
    key = ("nc", apply_gb)
    if key not in _CACHE:
        _CACHE[key] = _build(apply_gb)
    nc = _CACHE[key]
    in_maps = _prep_inputs(**inputs)
    res = run_bass_kernel_spmd(nc, in_maps, list(range(N_CORES)),
                               trace=trace)
    out = np.empty((B, S, HID), np.float32)
    for c in range(N_CORES):
        b, h = c // 2, c % 2
        o = res.results[c]["out"]
        for k in range(8):
            out[b, 256 * k + 128 * h:256 * k + 128 * h + 128, :] = \
                o[128 * k:128 * (k + 1), :]
    return out, res


def kernel(**inputs):
    out, _ = _run(inputs, trace=False)
    return out
